# revision 8
# baseline (speedup 1.0000x reference)
"""Causal multi-head attention (nn_Attention_87840671138123) on 8 trn2 NeuronCores.

Problem (B=2, S=2048, D=1024, H=16 heads, E=64 head_dim), fp32:
    Q = einsum('bsd,hde->bhse', q, W_q)   (same for K, V)
    scores = Q @ K^T / sqrt(D), causal mask, softmax
    attn = probs @ V  -> [B, S, D] (head-major concat)
    out = attn @ W_o.T

Sharding: core = 4*b + quad. Each core handles batch b and a quad of 4 heads
(heads 4*quad .. 4*quad+3). It computes a partial output
    out_part = attn_quad @ W_o.T[quad rows, :]   [S, D]
and the host sums the 4 partials per batch (the "all-reduce" of the output
projection done host-side at gather time).

Device layout choices (per core):
 - Host passes xT = x[b].T  [D, S] so the d-contraction sits on partitions.
 - Projections produce QT/KT in "transposed" layout [head-pair x 64, S]
   (head h2 of a pair occupies partitions 64*h2..64*h2+63), and V in natural
   [t, e] layout augmented with a ones-column (V_aug [t, 65]) so the
   attn matmul also accumulates the softmax denominator as row 64.
 - scoresT[t, s] = (KT chunk).T @ QT  -> exp on ACT (scale 1/32 folded in)
   -> causal handled by (a) skipping fully-masked blocks, (b) shrinking the
   moving dim to the valid s-range for diagonal blocks, (c) one [128,128]
   triangular mask multiply for the diagonal 128-col strip.
 - attnT_aug[65, s] += V_aug.T @ expT accumulated over t chunks in PSUM.
 - Normalize: denom row -> reciprocal -> partition_broadcast -> multiply.
 - Output projection: out[s, :] = sum_g (attnT chunk).T @ W_o.T slice.

All matmuls run in float32r (tf32-like, ~1.6e-4 rel err, full PE speed).
"""

import numpy as np

import concourse.bass as bass
import concourse.tile as tile
from concourse import bacc, mybir
from concourse.bass_utils import run_bass_kernel_spmd

B, S, D, H, E = 2, 2048, 1024, 16, 64
P = 128
NCORES = 8
SJ = 512            # s-tile width
NJ = S // SJ        # 4 s-tiles
ND = D // P         # 8 d-chunks
NT = S // P         # 16 t-chunks
f32 = mybir.dt.float32
f32r = mybir.dt.float32r
EXP = mybir.ActivationFunctionType.Exp
MULT = mybir.AluOpType.mult

_NC_CACHE = []


def _build():
    nc = bacc.Bacc("TRN2", target_bir_lowering=False, debug=False)

    qT_d = nc.dram_tensor("qT", [D, S], f32r, kind="ExternalInput")
    kT_d = nc.dram_tensor("kT", [D, S], f32r, kind="ExternalInput")
    vT_d = nc.dram_tensor("vT", [D, S], f32r, kind="ExternalInput")
    wq_d = nc.dram_tensor("wq", [D, 4 * E], f32r, kind="ExternalInput")
    wk_d = nc.dram_tensor("wk", [D, 4 * E], f32r, kind="ExternalInput")
    wv_d = nc.dram_tensor("wv", [D, 4 * E], f32r, kind="ExternalInput")
    wot_d = nc.dram_tensor("wot", [4 * E, D], f32r, kind="ExternalInput")
    tri_d = nc.dram_tensor("tri", [P, P], f32r, kind="ExternalInput")
    out_d = nc.dram_tensor("out", [S, D], f32, kind="ExternalOutput")

    with tile.TileContext(nc) as tc:
        with (
            tc.tile_pool(name="pers", bufs=1) as pers,
            tc.tile_pool(name="xt", bufs=3) as xt_pool,
            tc.tile_pool(name="ex", bufs=3) as ex_pool,
            tc.tile_pool(name="sm", bufs=3) as sm_pool,
            tc.tile_pool(name="ot", bufs=2) as ot_pool,
            tc.tile_pool(name="pj", bufs=2, space="PSUM") as pj_pool,
            tc.tile_pool(name="sc", bufs=3, space="PSUM") as sc_pool,
            tc.tile_pool(name="at", bufs=2, space="PSUM") as at_pool,
            tc.tile_pool(name="wm", bufs=1, space="PSUM") as wm_pool,
        ):
            # ---- persistent weights / constants ----
            wq_sb = pers.tile([P, ND, 4 * E], f32r, name="wq_sb")
            wk_sb = pers.tile([P, ND, 4 * E], f32r, name="wk_sb")
            wv_sb = pers.tile([P, ND, 4 * E], f32r, name="wv_sb")
            nc.sync.dma_start(wq_sb[:], wq_d.ap().rearrange("(o p) m -> p o m", p=P))
            nc.sync.dma_start(wk_sb[:], wk_d.ap().rearrange("(o p) m -> p o m", p=P))
            nc.sync.dma_start(wv_sb[:], wv_d.ap().rearrange("(o p) m -> p o m", p=P))
            wot_sb = pers.tile([P, 2, D], f32r, name="wot_sb")
            nc.sync.dma_start(wot_sb[:], wot_d.ap().rearrange("(g p) n -> p g n", p=P))
            tri_sb = pers.tile([P, P], f32r, name="tri_sb")
            nc.sync.dma_start(tri_sb[:], tri_d.ap())

            # Tiny bf16 constant operands for "HAM warmer" matmuls: fp32r
            # matmuls don't register as PE activity, so the HAM clock gate
            # re-throttles to 1.2 GHz mid-kernel. A ~60ns bf16 matmul
            # sprinkled into the PE stream keeps K=8/8 (2.4 GHz).
            warm_sb = pers.tile([P, 64], mybir.dt.bfloat16, name="warm_sb")
            nc.vector.memset(warm_sb[:], 0.0)

            # ---- persistent activations ----
            QT = [pers.tile([P, S], f32r, name=f"QT{g}") for g in range(2)]
            KT = [pers.tile([P, S], f32r, name=f"KT{g}") for g in range(2)]
            # V_aug: [t partition, t-chunk, head-in-pair, 64 V cols + ones col]
            V = [pers.tile([P, NT, 2, E + 1], f32r, name=f"V{g}") for g in range(2)]
            attnG = [pers.tile([P, S], f32r, name=f"attnG{g}") for g in range(2)]
            for g in range(2):
                nc.vector.memset(V[g][:, :, :, E:E + 1].bitcast(f32), 1.0)

            # ---- phase 1: projections ----
            for j in range(NJ):
                js = slice(j * SJ, (j + 1) * SJ)
                xq = xt_pool.tile([P, ND, SJ], f32r, tag="xt", name=f"xq{j}")
                nc.sync.dma_start(
                    xq[:], qT_d.ap().rearrange("(o p) s -> p o s", p=P)[:, :, js])
                for g in range(2):
                    wmq = wm_pool.tile([32, 64], f32, tag="wm", name=f"wmq{j}{g}")
                    nc.tensor.matmul(wmq[:], warm_sb[:, 0:32], warm_sb[:],
                                     start=True, stop=True)
                    pq = pj_pool.tile([P, SJ], f32, tag="pj", name=f"pq{j}{g}")
                    for c in range(ND):
                        nc.tensor.matmul(
                            pq[:], wq_sb[:, c, bass.ts(g, P)], xq[:, c, :],
                            start=(c == 0), stop=(c == ND - 1))
                    nc.vector.tensor_copy(QT[g][:, js], pq[:])

                xk = xt_pool.tile([P, ND, SJ], f32r, tag="xt", name=f"xk{j}")
                nc.sync.dma_start(
                    xk[:], kT_d.ap().rearrange("(o p) s -> p o s", p=P)[:, :, js])
                for g in range(2):
                    wmk = wm_pool.tile([32, 64], f32, tag="wm", name=f"wmk{j}{g}")
                    nc.tensor.matmul(wmk[:], warm_sb[:, 0:32], warm_sb[:],
                                     start=True, stop=True)
                    pk = pj_pool.tile([P, SJ], f32, tag="pj", name=f"pk{j}{g}")
                    for c in range(ND):
                        nc.tensor.matmul(
                            pk[:], wk_sb[:, c, bass.ts(g, P)], xk[:, c, :],
                            start=(c == 0), stop=(c == ND - 1))
                    nc.vector.tensor_copy(KT[g][:, js], pk[:])

                xv = xt_pool.tile([P, ND, SJ], f32r, tag="xt", name=f"xv{j}")
                nc.sync.dma_start(
                    xv[:], vT_d.ap().rearrange("(o p) s -> p o s", p=P)[:, :, js])
                for g in range(2):
                    for u in range(SJ // P):
                        t = 4 * j + u
                        wmv = wm_pool.tile([32, 64], f32, tag="wm",
                                           name=f"wmv{j}{g}{u}")
                        nc.tensor.matmul(wmv[:], warm_sb[:, 0:32], warm_sb[:],
                                         start=True, stop=True)
                        pv = pj_pool.tile([P, P], f32, tag="pj", name=f"pv{j}{g}{u}")
                        for c in range(ND):
                            nc.tensor.matmul(
                                pv[:], xv[:, c, bass.ts(u, P)],
                                wv_sb[:, c, bass.ts(g, P)],
                                start=(c == 0), stop=(c == ND - 1))
                        nc.vector.tensor_copy(V[g][:, t, 0, 0:E], pv[:, 0:E])
                        nc.vector.tensor_copy(V[g][:, t, 1, 0:E], pv[:, E:2 * E])

            # ---- phase 2: attention per (pair g, s-tile j) ----
            for g in range(2):
                for j in range(NJ):
                    nblk = 4 * j + 4
                    atp = [
                        at_pool.tile([P, SJ], f32, tag="at", name=f"at{g}{j}{h2}")
                        for h2 in range(2)
                    ]
                    for cb in range(nblk):
                        col0 = max(0, cb - 4 * j) * P
                        # HAM warmer: keep the PE clock gate at 8/8
                        wmp = wm_pool.tile([32, 64], f32, tag="wm",
                                           name=f"wm{g}{j}{cb}")
                        nc.tensor.matmul(wmp[:], warm_sb[:, 0:32], warm_sb[:],
                                         start=True, stop=True)
                        # both heads' score matmuls back to back: K=64 row
                        # groups (0,*) and (64,*) run concurrently on the PE
                        scps = []
                        for h2 in range(2):
                            hs = slice(E * h2, E * h2 + E)
                            scp = sc_pool.tile(
                                [P, SJ], f32, tag="sc", name=f"sc{g}{j}{cb}{h2}")
                            nc.tensor.matmul(
                                scp[:, col0:],
                                KT[g][hs, bass.ts(cb, P)],
                                QT[g][hs, j * SJ + col0:(j + 1) * SJ],
                                start=True, stop=True)
                            scps.append(scp)
                        for h2 in range(2):
                            scp = scps[h2]
                            ex = ex_pool.tile(
                                [P, SJ], f32r, tag="ex", name=f"ex{g}{j}{cb}{h2}")
                            nc.scalar.activation(
                                ex[:, col0:], scp[:, col0:], EXP, scale=1.0 / 32.0)
                            if cb >= 4 * j:
                                nc.vector.tensor_tensor(
                                    ex[:, col0:col0 + P], ex[:, col0:col0 + P],
                                    tri_sb[:], MULT)
                            nc.tensor.matmul(
                                atp[h2][0:E + 1, col0:],
                                V[g][:, cb, h2, :],
                                ex[:, col0:],
                                start=(cb == 0), stop=(cb == nblk - 1))
                    # epilogue: normalize by softmax denominator (row E)
                    for h2 in range(2):
                        js = slice(j * SJ, (j + 1) * SJ)
                        den = sm_pool.tile([E + 1, SJ], f32, tag="den",
                                           name=f"den{g}{j}{h2}")
                        nc.vector.tensor_copy(den[E:E + 1, :], atp[h2][E:E + 1, :])
                        rec = sm_pool.tile([1, SJ], f32, tag="rec",
                                           name=f"rec{g}{j}{h2}")
                        nc.sync.dma_start(rec[:], den[E:E + 1, :])
                        nc.vector.reciprocal(rec[:], rec[:])
                        recb = sm_pool.tile([E, SJ], f32, tag="recb",
                                            name=f"recb{g}{j}{h2}")
                        nc.gpsimd.partition_broadcast(recb[:], rec[:])
                        if h2 == 0:
                            nc.vector.tensor_tensor(
                                attnG[g][0:E, js], atp[h2][0:E, :], recb[:], MULT)
                        else:
                            ah = sm_pool.tile([E, SJ], f32r, tag="ah",
                                              name=f"ah{g}{j}")
                            nc.vector.tensor_tensor(
                                ah[:], atp[h2][0:E, :], recb[:], MULT)
                            nc.sync.dma_start(attnG[g][E:2 * E, js], ah[:])

            # ---- phase 3: output projection (partial over this core's heads) --
            for si in range(NT):
                wmo = wm_pool.tile([32, 64], f32, tag="wm", name=f"wmo{si}")
                nc.tensor.matmul(wmo[:], warm_sb[:, 0:32], warm_sb[:],
                                 start=True, stop=True)
                ot = ot_pool.tile([P, D], f32, tag="ot", name=f"ot{si}")
                for no in range(2):
                    po = pj_pool.tile([P, SJ], f32, tag="pj", name=f"po{si}{no}")
                    for g in range(2):
                        nc.tensor.matmul(
                            po[:], attnG[g][:, bass.ts(si, P)],
                            wot_sb[:, g, bass.ts(no, SJ)],
                            start=(g == 0), stop=(g == 1))
                    nc.vector.tensor_copy(ot[:, bass.ts(no, SJ)], po[:])
                nc.sync.dma_start(out_d.ap()[bass.ts(si, P), :], ot[:])

    nc.compile()
    return nc


def _get_nc():
    if not _NC_CACHE:
        _NC_CACHE.append(_build())
    return _NC_CACHE[0]


def _in_maps(q, k, v, W_q, W_k, W_v, W_o):
    tri = (np.arange(P)[:, None] <= np.arange(P)[None, :]).astype(np.float32)
    xT = {}
    for b in range(B):
        xT[b] = tuple(
            np.ascontiguousarray(x[b].T) for x in (q, k, v))
    maps = []
    for core in range(NCORES):
        b, quad = divmod(core, 4)
        hs = slice(4 * quad, 4 * quad + 4)
        qT_b, kT_b, vT_b = xT[b]
        maps.append({
            "qT": qT_b,
            "kT": kT_b,
            "vT": vT_b,
            # [4, D, E] -> [D, 4, E] -> [D, 256], col l*64+e = W[4q+l, d, e]
            "wq": np.ascontiguousarray(
                W_q[hs].transpose(1, 0, 2).reshape(D, 4 * E)),
            "wk": np.ascontiguousarray(
                W_k[hs].transpose(1, 0, 2).reshape(D, 4 * E)),
            "wv": np.ascontiguousarray(
                W_v[hs].transpose(1, 0, 2).reshape(D, 4 * E)),
            # W_o[out, in] -> W_o.T rows for this quad's 256 input dims
            "wot": np.ascontiguousarray(
                W_o[:, 4 * quad * E:4 * quad * E + 4 * E].T),
            "tri": tri,
        })
    return maps


def kernel(q, k, v, W_q, W_k, W_v, W_o, _trace=False, _trace_kwargs=None):
    q = np.asarray(q, dtype=np.float32)
    k = np.asarray(k, dtype=np.float32)
    v = np.asarray(v, dtype=np.float32)
    W_q = np.asarray(W_q, dtype=np.float32)
    W_k = np.asarray(W_k, dtype=np.float32)
    W_v = np.asarray(W_v, dtype=np.float32)
    W_o = np.asarray(W_o, dtype=np.float32)

    nc = _get_nc()
    maps = _in_maps(q, k, v, W_q, W_k, W_v, W_o)
    kwargs = dict(_trace_kwargs or {})
    res = run_bass_kernel_spmd(
        nc, maps, core_ids=list(range(NCORES)), trace=_trace, **kwargs)
    out = np.zeros((B, S, D), dtype=np.float32)
    for core in range(NCORES):
        b = core // 4
        out[b] += res.results[core]["out"]
    if _trace:
        kernel.last_results = res
    return out


# revision 9
# speedup vs baseline: 1.1685x; 1.1685x over previous
"""Causal multi-head attention (nn_Attention_87840671138123) on 8 trn2 NeuronCores.

Problem (B=2, S=2048, D=1024, H=16 heads, E=64 head_dim), fp32:
    Q = einsum('bsd,hde->bhse', q, W_q)   (same for K, V)
    scores = Q @ K^T / sqrt(D), causal mask, softmax
    attn = probs @ V  -> [B, S, D] (head-major concat)
    out = attn @ W_o.T

Sharding: core = 4*b + quad. Each core handles batch b and a quad of 4 heads
(heads 4*quad .. 4*quad+3). It computes a partial output
    out_part = attn_quad @ W_o.T[quad rows, :]   [S, D]
and the host sums the 4 partials per batch (the "all-reduce" of the output
projection done host-side at gather time).

Device layout choices (per core):
 - Host passes xT = x[b].T  [D, S] so the d-contraction sits on partitions.
 - Projections produce QT/KT in "transposed" layout [head-pair x 64, S]
   (head h2 of a pair occupies partitions 64*h2..64*h2+63), and V in natural
   [t, e] layout augmented with a ones-column (V_aug [t, 65]) so the
   attn matmul also accumulates the softmax denominator as row 64.
 - scoresT[t, s] = (KT chunk).T @ QT  -> exp on ACT (scale 1/32 folded in)
   -> causal handled by (a) skipping fully-masked blocks, (b) shrinking the
   moving dim to the valid s-range for diagonal blocks, (c) one [128,128]
   triangular mask multiply for the diagonal 128-col strip.
 - attnT_aug[65, s] += V_aug.T @ expT accumulated over t chunks in PSUM.
 - Normalize: denom row -> reciprocal -> partition_broadcast -> multiply.
 - Output projection: out[s, :] = sum_g (attnT chunk).T @ W_o.T slice.

Numerics: the Q/K path (projections + scores) runs in bf16 — score errors
are absolute-small (scores ~N(0, 0.1^2)) and only perturb softmax weights,
contributing <~3e-4 relative to the output. The V path (V projection,
attn*V, W_o) stays float32r (tf32-like): value errors there pass straight
through to the output. fp32r also runs at a lower power draw than fp32
HIGH-mode; the all-fp32r version tripped the chip-wide power throttle
(all 8 cores pinned at K=4/8 = 1.2 GHz for ~270us).
"""

import ml_dtypes
import numpy as np

import concourse.bass as bass
import concourse.tile as tile
from concourse import bacc, mybir
from concourse.bass_utils import run_bass_kernel_spmd

B, S, D, H, E = 2, 2048, 1024, 16, 64
P = 128
NCORES = 8
SJ = 512            # s-tile width
NJ = S // SJ        # 4 s-tiles
ND = D // P         # 8 d-chunks
NT = S // P         # 16 t-chunks
f32 = mybir.dt.float32
f32r = mybir.dt.float32r
bf16 = mybir.dt.bfloat16
EXP = mybir.ActivationFunctionType.Exp
MULT = mybir.AluOpType.mult

QK_DT = bf16        # dtype of q/k inputs, Wq/Wk, QT/KT, scores matmul
V_DT = f32r         # dtype of v input, Wv, V_aug, expT, attnG, WoT

_NP_OF = {bf16: ml_dtypes.bfloat16, f32r: np.float32, f32: np.float32}

_NC_CACHE = []


def _build():
    nc = bacc.Bacc("TRN2", target_bir_lowering=False, debug=False)

    qT_d = nc.dram_tensor("qT", [D, S], QK_DT, kind="ExternalInput")
    kT_d = nc.dram_tensor("kT", [D, S], QK_DT, kind="ExternalInput")
    vT_d = nc.dram_tensor("vT", [D, S], V_DT, kind="ExternalInput")
    wq_d = nc.dram_tensor("wq", [D, 4 * E], QK_DT, kind="ExternalInput")
    wk_d = nc.dram_tensor("wk", [D, 4 * E], QK_DT, kind="ExternalInput")
    wv_d = nc.dram_tensor("wv", [D, 4 * E], V_DT, kind="ExternalInput")
    wot_d = nc.dram_tensor("wot", [4 * E, D], V_DT, kind="ExternalInput")
    tri_d = nc.dram_tensor("tri", [P, P], V_DT, kind="ExternalInput")
    out_d = nc.dram_tensor("out", [S, D], f32, kind="ExternalOutput")

    with tile.TileContext(nc) as tc:
        with (
            tc.tile_pool(name="pers", bufs=1) as pers,
            tc.tile_pool(name="xt", bufs=3) as xt_pool,
            tc.tile_pool(name="ex", bufs=3) as ex_pool,
            tc.tile_pool(name="sm", bufs=3) as sm_pool,
            tc.tile_pool(name="ot", bufs=2) as ot_pool,
            tc.tile_pool(name="pj", bufs=3, space="PSUM") as pj_pool,
            tc.tile_pool(name="sc", bufs=3, space="PSUM") as sc_pool,
            tc.tile_pool(name="at", bufs=2, space="PSUM") as at_pool,
        ):
            # ---- persistent weights / constants ----
            wq_sb = pers.tile([P, ND, 4 * E], QK_DT, name="wq_sb")
            wk_sb = pers.tile([P, ND, 4 * E], QK_DT, name="wk_sb")
            wv_sb = pers.tile([P, ND, 4 * E], V_DT, name="wv_sb")
            nc.sync.dma_start(wq_sb[:], wq_d.ap().rearrange("(o p) m -> p o m", p=P))
            nc.sync.dma_start(wk_sb[:], wk_d.ap().rearrange("(o p) m -> p o m", p=P))
            nc.sync.dma_start(wv_sb[:], wv_d.ap().rearrange("(o p) m -> p o m", p=P))
            wot_sb = pers.tile([P, 2, D], V_DT, name="wot_sb")
            nc.sync.dma_start(wot_sb[:], wot_d.ap().rearrange("(g p) n -> p g n", p=P))
            tri_sb = pers.tile([P, P], V_DT, name="tri_sb")
            nc.sync.dma_start(tri_sb[:], tri_d.ap())

            # ---- persistent activations ----
            QT = [pers.tile([P, S], QK_DT, name=f"QT{g}") for g in range(2)]
            KT = [pers.tile([P, S], QK_DT, name=f"KT{g}") for g in range(2)]
            # V_aug: [t partition, t-chunk, head-in-pair, 64 V cols + ones col]
            V = [pers.tile([P, NT, 2, E + 1], V_DT, name=f"V{g}") for g in range(2)]
            attnG = [pers.tile([P, S], V_DT, name=f"attnG{g}") for g in range(2)]
            for g in range(2):
                nc.vector.memset(V[g][:, :, :, E:E + 1].bitcast(f32), 1.0)

            # ---- phase 1: projections ----
            for j in range(NJ):
                js = slice(j * SJ, (j + 1) * SJ)
                xq = xt_pool.tile([P, ND, SJ], QK_DT, tag="xtq", name=f"xq{j}")
                nc.sync.dma_start(
                    xq[:], qT_d.ap().rearrange("(o p) s -> p o s", p=P)[:, :, js])
                for g in range(2):
                    pq = pj_pool.tile([P, SJ], f32, tag="pj", name=f"pq{j}{g}")
                    for c in range(ND):
                        nc.tensor.matmul(
                            pq[:], wq_sb[:, c, bass.ts(g, P)], xq[:, c, :],
                            start=(c == 0), stop=(c == ND - 1))
                    nc.vector.tensor_copy(QT[g][:, js], pq[:])

                xk = xt_pool.tile([P, ND, SJ], QK_DT, tag="xtq", name=f"xk{j}")
                nc.sync.dma_start(
                    xk[:], kT_d.ap().rearrange("(o p) s -> p o s", p=P)[:, :, js])
                for g in range(2):
                    pk = pj_pool.tile([P, SJ], f32, tag="pj", name=f"pk{j}{g}")
                    for c in range(ND):
                        nc.tensor.matmul(
                            pk[:], wk_sb[:, c, bass.ts(g, P)], xk[:, c, :],
                            start=(c == 0), stop=(c == ND - 1))
                    nc.vector.tensor_copy(KT[g][:, js], pk[:])

                xv = xt_pool.tile([P, ND, SJ], V_DT, tag="xtv", name=f"xv{j}")
                nc.sync.dma_start(
                    xv[:], vT_d.ap().rearrange("(o p) s -> p o s", p=P)[:, :, js])
                for g in range(2):
                    for u in range(SJ // P):
                        t = 4 * j + u
                        pv = pj_pool.tile([P, P], f32, tag="pj", name=f"pv{j}{g}{u}")
                        for c in range(ND):
                            nc.tensor.matmul(
                                pv[:], xv[:, c, bass.ts(u, P)],
                                wv_sb[:, c, bass.ts(g, P)],
                                start=(c == 0), stop=(c == ND - 1))
                        nc.vector.tensor_copy(V[g][:, t, 0, 0:E], pv[:, 0:E])
                        nc.vector.tensor_copy(V[g][:, t, 1, 0:E], pv[:, E:2 * E])

            # ---- phase 2: attention per (pair g, s-tile j) ----
            for g in range(2):
                for j in range(NJ):
                    nblk = 4 * j + 4
                    atp = [
                        at_pool.tile([P, SJ], f32, tag="at", name=f"at{g}{j}{h2}")
                        for h2 in range(2)
                    ]
                    for cb in range(nblk):
                        col0 = max(0, cb - 4 * j) * P
                        # both heads' score matmuls back to back: K=64 row
                        # groups (0,*) and (64,*) run concurrently on the PE
                        scps = []
                        for h2 in range(2):
                            hs = slice(E * h2, E * h2 + E)
                            scp = sc_pool.tile(
                                [P, SJ], f32, tag="sc", name=f"sc{g}{j}{cb}{h2}")
                            nc.tensor.matmul(
                                scp[:, col0:],
                                KT[g][hs, bass.ts(cb, P)],
                                QT[g][hs, j * SJ + col0:(j + 1) * SJ],
                                start=True, stop=True)
                            scps.append(scp)
                        for h2 in range(2):
                            scp = scps[h2]
                            ex = ex_pool.tile(
                                [P, SJ], V_DT, tag="ex", name=f"ex{g}{j}{cb}{h2}")
                            nc.scalar.activation(
                                ex[:, col0:], scp[:, col0:], EXP, scale=1.0 / 32.0)
                            if cb >= 4 * j:
                                nc.vector.tensor_tensor(
                                    ex[:, col0:col0 + P], ex[:, col0:col0 + P],
                                    tri_sb[:], MULT)
                            nc.tensor.matmul(
                                atp[h2][0:E + 1, col0:],
                                V[g][:, cb, h2, :],
                                ex[:, col0:],
                                start=(cb == 0), stop=(cb == nblk - 1))
                    # epilogue: normalize by softmax denominator (row E)
                    for h2 in range(2):
                        js = slice(j * SJ, (j + 1) * SJ)
                        den = sm_pool.tile([E + 1, SJ], f32, tag="den",
                                           name=f"den{g}{j}{h2}")
                        nc.vector.tensor_copy(den[E:E + 1, :], atp[h2][E:E + 1, :])
                        rec = sm_pool.tile([1, SJ], f32, tag="rec",
                                           name=f"rec{g}{j}{h2}")
                        nc.sync.dma_start(rec[:], den[E:E + 1, :])
                        nc.vector.reciprocal(rec[:], rec[:])
                        recb = sm_pool.tile([E, SJ], f32, tag="recb",
                                            name=f"recb{g}{j}{h2}")
                        nc.gpsimd.partition_broadcast(recb[:], rec[:])
                        if h2 == 0:
                            nc.vector.tensor_tensor(
                                attnG[g][0:E, js], atp[h2][0:E, :], recb[:], MULT)
                        else:
                            ah = sm_pool.tile([E, SJ], V_DT, tag="ah",
                                              name=f"ah{g}{j}")
                            nc.vector.tensor_tensor(
                                ah[:], atp[h2][0:E, :], recb[:], MULT)
                            nc.sync.dma_start(attnG[g][E:2 * E, js], ah[:])

            # ---- phase 3: output projection (partial over this core's heads) --
            for si in range(NT):
                ot = ot_pool.tile([P, D], f32, tag="ot", name=f"ot{si}")
                for no in range(2):
                    po = pj_pool.tile([P, SJ], f32, tag="pj", name=f"po{si}{no}")
                    for g in range(2):
                        nc.tensor.matmul(
                            po[:], attnG[g][:, bass.ts(si, P)],
                            wot_sb[:, g, bass.ts(no, SJ)],
                            start=(g == 0), stop=(g == 1))
                    nc.vector.tensor_copy(ot[:, bass.ts(no, SJ)], po[:])
                nc.sync.dma_start(out_d.ap()[bass.ts(si, P), :], ot[:])

    nc.compile()
    return nc


def _get_nc():
    if not _NC_CACHE:
        _NC_CACHE.append(_build())
    return _NC_CACHE[0]


def _in_maps(q, k, v, W_q, W_k, W_v, W_o):
    qk_np = _NP_OF[QK_DT]
    v_np = _NP_OF[V_DT]
    tri = (np.arange(P)[:, None] <= np.arange(P)[None, :]).astype(v_np)
    xT = {}
    for b in range(B):
        xT[b] = (
            np.ascontiguousarray(q[b].T).astype(qk_np),
            np.ascontiguousarray(k[b].T).astype(qk_np),
            np.ascontiguousarray(v[b].T).astype(v_np),
        )
    maps = []
    for core in range(NCORES):
        b, quad = divmod(core, 4)
        hs = slice(4 * quad, 4 * quad + 4)
        qT_b, kT_b, vT_b = xT[b]
        maps.append({
            "qT": qT_b,
            "kT": kT_b,
            "vT": vT_b,
            # [4, D, E] -> [D, 4, E] -> [D, 256], col l*64+e = W[4q+l, d, e]
            "wq": np.ascontiguousarray(
                W_q[hs].transpose(1, 0, 2).reshape(D, 4 * E)).astype(qk_np),
            "wk": np.ascontiguousarray(
                W_k[hs].transpose(1, 0, 2).reshape(D, 4 * E)).astype(qk_np),
            "wv": np.ascontiguousarray(
                W_v[hs].transpose(1, 0, 2).reshape(D, 4 * E)).astype(v_np),
            # W_o[out, in] -> W_o.T rows for this quad's 256 input dims
            "wot": np.ascontiguousarray(
                W_o[:, 4 * quad * E:4 * quad * E + 4 * E].T).astype(v_np),
            "tri": tri,
        })
    return maps


def kernel(q, k, v, W_q, W_k, W_v, W_o, _trace=False, _trace_kwargs=None):
    q = np.asarray(q, dtype=np.float32)
    k = np.asarray(k, dtype=np.float32)
    v = np.asarray(v, dtype=np.float32)
    W_q = np.asarray(W_q, dtype=np.float32)
    W_k = np.asarray(W_k, dtype=np.float32)
    W_v = np.asarray(W_v, dtype=np.float32)
    W_o = np.asarray(W_o, dtype=np.float32)

    nc = _get_nc()
    maps = _in_maps(q, k, v, W_q, W_k, W_v, W_o)
    kwargs = dict(_trace_kwargs or {})
    res = run_bass_kernel_spmd(
        nc, maps, core_ids=list(range(NCORES)), trace=_trace, **kwargs)
    out = np.zeros((B, S, D), dtype=np.float32)
    for core in range(NCORES):
        b = core // 4
        out[b] += res.results[core]["out"]
    if _trace:
        kernel.last_results = res
    return out


# revision 10
# speedup vs baseline: 1.2908x; 1.1046x over previous
"""Causal multi-head attention (nn_Attention_87840671138123) on 8 trn2 NeuronCores.

Problem (B=2, S=2048, D=1024, H=16 heads, E=64 head_dim), fp32:
    Q = einsum('bsd,hde->bhse', q, W_q)   (same for K, V)
    scores = Q @ K^T / sqrt(D), causal mask, softmax
    attn = probs @ V  -> [B, S, D] (head-major concat)
    out = attn @ W_o.T

Sharding: core = 4*b + quad. Each core handles batch b and a quad of 4 heads
(heads 4*quad .. 4*quad+3). It computes a partial output
    out_part = attn_quad @ W_o.T[quad rows, :]   [S, D]
and the host sums the 4 partials per batch (the "all-reduce" of the output
projection done host-side at gather time).

Device layout choices (per core):
 - Host passes xT = x[b].T  [D, S] so the d-contraction sits on partitions.
 - Projections produce QT/KT in "transposed" layout [head-pair x 64, S]
   (head h2 of a pair occupies partitions 64*h2..64*h2+63), and V in natural
   [t, e] layout augmented with a ones-column (V_aug [t, 65]) so the
   attn matmul also accumulates the softmax denominator as row 64.
 - scoresT[t, s] = (KT chunk).T @ QT  -> exp on ACT (scale 1/32 folded in)
   -> causal handled by (a) skipping fully-masked blocks, (b) shrinking the
   moving dim to the valid s-range for diagonal blocks, (c) one [128,128]
   triangular mask multiply for the diagonal 128-col strip.
 - attnT_aug[65, s] += V_aug.T @ expT accumulated over t chunks in PSUM.
 - Normalize: denom row -> reciprocal -> partition_broadcast -> multiply.
 - Output projection: out[s, :] = sum_g (attnT chunk).T @ W_o.T slice.

Numerics: the Q/K path (projections + scores) runs in bf16 — score errors
are absolute-small (scores ~N(0, 0.1^2)) and only perturb softmax weights,
contributing <~3e-4 relative to the output. The V path (V projection,
attn*V, W_o) stays float32r (tf32-like): value errors there pass straight
through to the output. fp32r also runs at a lower power draw than fp32
HIGH-mode; the all-fp32r version tripped the chip-wide power throttle
(all 8 cores pinned at K=4/8 = 1.2 GHz for ~270us).
"""

import ml_dtypes
import numpy as np

import concourse.bass as bass
import concourse.tile as tile
from concourse import bacc, mybir
from concourse.bass_utils import run_bass_kernel_spmd

B, S, D, H, E = 2, 2048, 1024, 16, 64
P = 128
NCORES = 8
SJ = 512            # s-tile width
NJ = S // SJ        # 4 s-tiles
ND = D // P         # 8 d-chunks
NT = S // P         # 16 t-chunks
f32 = mybir.dt.float32
f32r = mybir.dt.float32r
bf16 = mybir.dt.bfloat16
EXP = mybir.ActivationFunctionType.Exp
MULT = mybir.AluOpType.mult

QK_DT = bf16        # dtype of q/k inputs, Wq/Wk, QT/KT, scores matmul
V_DT = bf16         # dtype of v input, Wv, V_aug, expT, attnG, WoT

_NP_OF = {bf16: ml_dtypes.bfloat16, f32r: np.float32, f32: np.float32}

_NC_CACHE = []


def _build():
    nc = bacc.Bacc("TRN2", target_bir_lowering=False, debug=False)

    qT_d = nc.dram_tensor("qT", [D, S], QK_DT, kind="ExternalInput")
    kT_d = nc.dram_tensor("kT", [D, S], QK_DT, kind="ExternalInput")
    vT_d = nc.dram_tensor("vT", [D, S], V_DT, kind="ExternalInput")
    wq_d = nc.dram_tensor("wq", [D, 4 * E], QK_DT, kind="ExternalInput")
    wk_d = nc.dram_tensor("wk", [D, 4 * E], QK_DT, kind="ExternalInput")
    wv_d = nc.dram_tensor("wv", [D, 4 * E], V_DT, kind="ExternalInput")
    wot_d = nc.dram_tensor("wot", [4 * E, D], V_DT, kind="ExternalInput")
    tri_d = nc.dram_tensor("tri", [P, P], V_DT, kind="ExternalInput")
    out_d = nc.dram_tensor("out", [S, D], f32, kind="ExternalOutput")

    with tile.TileContext(nc) as tc:
        with (
            tc.tile_pool(name="pers", bufs=1) as pers,
            tc.tile_pool(name="xt", bufs=3) as xt_pool,
            tc.tile_pool(name="ex", bufs=3) as ex_pool,
            tc.tile_pool(name="sm", bufs=3) as sm_pool,
            tc.tile_pool(name="ot", bufs=2) as ot_pool,
            tc.tile_pool(name="pj", bufs=3, space="PSUM") as pj_pool,
            tc.tile_pool(name="sc", bufs=3, space="PSUM") as sc_pool,
            tc.tile_pool(name="at", bufs=2, space="PSUM") as at_pool,
        ):
            # ---- persistent weights / constants ----
            wq_sb = pers.tile([P, ND, 4 * E], QK_DT, name="wq_sb")
            wk_sb = pers.tile([P, ND, 4 * E], QK_DT, name="wk_sb")
            wv_sb = pers.tile([P, ND, 4 * E], V_DT, name="wv_sb")
            nc.sync.dma_start(wq_sb[:], wq_d.ap().rearrange("(o p) m -> p o m", p=P))
            nc.sync.dma_start(wk_sb[:], wk_d.ap().rearrange("(o p) m -> p o m", p=P))
            nc.sync.dma_start(wv_sb[:], wv_d.ap().rearrange("(o p) m -> p o m", p=P))
            wot_sb = pers.tile([P, 2, D], V_DT, name="wot_sb")
            nc.sync.dma_start(wot_sb[:], wot_d.ap().rearrange("(g p) n -> p g n", p=P))
            tri_sb = pers.tile([P, P], V_DT, name="tri_sb")
            nc.sync.dma_start(tri_sb[:], tri_d.ap())

            # ---- persistent activations ----
            QT = [pers.tile([P, S], QK_DT, name=f"QT{g}") for g in range(2)]
            KT = [pers.tile([P, S], QK_DT, name=f"KT{g}") for g in range(2)]
            # V_aug: [t partition, t-chunk, head-in-pair, 64 V cols + ones col]
            V = [pers.tile([P, NT, 2, E + 1], V_DT, name=f"V{g}") for g in range(2)]
            attnG = [pers.tile([P, S], V_DT, name=f"attnG{g}") for g in range(2)]
            for g in range(2):
                ones_ap = V[g][:, :, :, E:E + 1]
                if V_DT == f32r:
                    ones_ap = ones_ap.bitcast(f32)
                nc.vector.memset(ones_ap, 1.0)

            # ---- phase 1: projections ----
            for j in range(NJ):
                js = slice(j * SJ, (j + 1) * SJ)
                xq = xt_pool.tile([P, ND, SJ], QK_DT, tag="xtq", name=f"xq{j}")
                nc.sync.dma_start(
                    xq[:], qT_d.ap().rearrange("(o p) s -> p o s", p=P)[:, :, js])
                for g in range(2):
                    pq = pj_pool.tile([P, SJ], f32, tag="pj", name=f"pq{j}{g}")
                    for c in range(ND):
                        nc.tensor.matmul(
                            pq[:], wq_sb[:, c, bass.ts(g, P)], xq[:, c, :],
                            start=(c == 0), stop=(c == ND - 1))
                    nc.vector.tensor_copy(QT[g][:, js], pq[:])

                xk = xt_pool.tile([P, ND, SJ], QK_DT, tag="xtq", name=f"xk{j}")
                nc.sync.dma_start(
                    xk[:], kT_d.ap().rearrange("(o p) s -> p o s", p=P)[:, :, js])
                for g in range(2):
                    pk = pj_pool.tile([P, SJ], f32, tag="pj", name=f"pk{j}{g}")
                    for c in range(ND):
                        nc.tensor.matmul(
                            pk[:], wk_sb[:, c, bass.ts(g, P)], xk[:, c, :],
                            start=(c == 0), stop=(c == ND - 1))
                    nc.vector.tensor_copy(KT[g][:, js], pk[:])

                xv = xt_pool.tile([P, ND, SJ], V_DT, tag="xtv", name=f"xv{j}")
                nc.sync.dma_start(
                    xv[:], vT_d.ap().rearrange("(o p) s -> p o s", p=P)[:, :, js])
                for g in range(2):
                    for u in range(SJ // P):
                        t = 4 * j + u
                        pv = pj_pool.tile([P, P], f32, tag="pj", name=f"pv{j}{g}{u}")
                        for c in range(ND):
                            nc.tensor.matmul(
                                pv[:], xv[:, c, bass.ts(u, P)],
                                wv_sb[:, c, bass.ts(g, P)],
                                start=(c == 0), stop=(c == ND - 1))
                        nc.vector.tensor_copy(V[g][:, t, 0, 0:E], pv[:, 0:E])
                        nc.vector.tensor_copy(V[g][:, t, 1, 0:E], pv[:, E:2 * E])

            # ---- phase 2: attention per (pair g, s-tile j) ----
            for g in range(2):
                for j in range(NJ):
                    nblk = 4 * j + 4
                    atp = [
                        at_pool.tile([P, SJ], f32, tag="at", name=f"at{g}{j}{h2}")
                        for h2 in range(2)
                    ]
                    for cb in range(nblk):
                        col0 = max(0, cb - 4 * j) * P
                        # both heads' score matmuls back to back: K=64 row
                        # groups (0,*) and (64,*) run concurrently on the PE
                        scps = []
                        for h2 in range(2):
                            hs = slice(E * h2, E * h2 + E)
                            scp = sc_pool.tile(
                                [P, SJ], f32, tag="sc", name=f"sc{g}{j}{cb}{h2}")
                            nc.tensor.matmul(
                                scp[:, col0:],
                                KT[g][hs, bass.ts(cb, P)],
                                QT[g][hs, j * SJ + col0:(j + 1) * SJ],
                                start=True, stop=True)
                            scps.append(scp)
                        for h2 in range(2):
                            scp = scps[h2]
                            ex = ex_pool.tile(
                                [P, SJ], V_DT, tag="ex", name=f"ex{g}{j}{cb}{h2}")
                            nc.scalar.activation(
                                ex[:, col0:], scp[:, col0:], EXP, scale=1.0 / 32.0)
                            if cb >= 4 * j:
                                nc.vector.tensor_tensor(
                                    ex[:, col0:col0 + P], ex[:, col0:col0 + P],
                                    tri_sb[:], MULT)
                            nc.tensor.matmul(
                                atp[h2][0:E + 1, col0:],
                                V[g][:, cb, h2, :],
                                ex[:, col0:],
                                start=(cb == 0), stop=(cb == nblk - 1))
                    # epilogue: normalize by softmax denominator (row E)
                    for h2 in range(2):
                        js = slice(j * SJ, (j + 1) * SJ)
                        den = sm_pool.tile([E + 1, SJ], f32, tag="den",
                                           name=f"den{g}{j}{h2}")
                        nc.vector.tensor_copy(den[E:E + 1, :], atp[h2][E:E + 1, :])
                        rec = sm_pool.tile([1, SJ], f32, tag="rec",
                                           name=f"rec{g}{j}{h2}")
                        nc.sync.dma_start(rec[:], den[E:E + 1, :])
                        nc.vector.reciprocal(rec[:], rec[:])
                        recb = sm_pool.tile([E, SJ], f32, tag="recb",
                                            name=f"recb{g}{j}{h2}")
                        nc.gpsimd.partition_broadcast(recb[:], rec[:])
                        if h2 == 0:
                            nc.vector.tensor_tensor(
                                attnG[g][0:E, js], atp[h2][0:E, :], recb[:], MULT)
                        else:
                            ah = sm_pool.tile([E, SJ], V_DT, tag="ah",
                                              name=f"ah{g}{j}")
                            nc.vector.tensor_tensor(
                                ah[:], atp[h2][0:E, :], recb[:], MULT)
                            nc.sync.dma_start(attnG[g][E:2 * E, js], ah[:])

            # ---- phase 3: output projection (partial over this core's heads) --
            for si in range(NT):
                ot = ot_pool.tile([P, D], f32, tag="ot", name=f"ot{si}")
                for no in range(2):
                    po = pj_pool.tile([P, SJ], f32, tag="pj", name=f"po{si}{no}")
                    for g in range(2):
                        nc.tensor.matmul(
                            po[:], attnG[g][:, bass.ts(si, P)],
                            wot_sb[:, g, bass.ts(no, SJ)],
                            start=(g == 0), stop=(g == 1))
                    nc.vector.tensor_copy(ot[:, bass.ts(no, SJ)], po[:])
                nc.sync.dma_start(out_d.ap()[bass.ts(si, P), :], ot[:])

    nc.compile()
    return nc


def _get_nc():
    if not _NC_CACHE:
        _NC_CACHE.append(_build())
    return _NC_CACHE[0]


def _in_maps(q, k, v, W_q, W_k, W_v, W_o):
    qk_np = _NP_OF[QK_DT]
    v_np = _NP_OF[V_DT]
    tri = (np.arange(P)[:, None] <= np.arange(P)[None, :]).astype(v_np)
    xT = {}
    for b in range(B):
        xT[b] = (
            np.ascontiguousarray(q[b].T).astype(qk_np),
            np.ascontiguousarray(k[b].T).astype(qk_np),
            np.ascontiguousarray(v[b].T).astype(v_np),
        )
    maps = []
    for core in range(NCORES):
        b, quad = divmod(core, 4)
        hs = slice(4 * quad, 4 * quad + 4)
        qT_b, kT_b, vT_b = xT[b]
        maps.append({
            "qT": qT_b,
            "kT": kT_b,
            "vT": vT_b,
            # [4, D, E] -> [D, 4, E] -> [D, 256], col l*64+e = W[4q+l, d, e]
            "wq": np.ascontiguousarray(
                W_q[hs].transpose(1, 0, 2).reshape(D, 4 * E)).astype(qk_np),
            "wk": np.ascontiguousarray(
                W_k[hs].transpose(1, 0, 2).reshape(D, 4 * E)).astype(qk_np),
            "wv": np.ascontiguousarray(
                W_v[hs].transpose(1, 0, 2).reshape(D, 4 * E)).astype(v_np),
            # W_o[out, in] -> W_o.T rows for this quad's 256 input dims
            "wot": np.ascontiguousarray(
                W_o[:, 4 * quad * E:4 * quad * E + 4 * E].T).astype(v_np),
            "tri": tri,
        })
    return maps


def kernel(q, k, v, W_q, W_k, W_v, W_o, _trace=False, _trace_kwargs=None):
    q = np.asarray(q, dtype=np.float32)
    k = np.asarray(k, dtype=np.float32)
    v = np.asarray(v, dtype=np.float32)
    W_q = np.asarray(W_q, dtype=np.float32)
    W_k = np.asarray(W_k, dtype=np.float32)
    W_v = np.asarray(W_v, dtype=np.float32)
    W_o = np.asarray(W_o, dtype=np.float32)

    nc = _get_nc()
    maps = _in_maps(q, k, v, W_q, W_k, W_v, W_o)
    kwargs = dict(_trace_kwargs or {})
    res = run_bass_kernel_spmd(
        nc, maps, core_ids=list(range(NCORES)), trace=_trace, **kwargs)
    out = np.zeros((B, S, D), dtype=np.float32)
    for core in range(NCORES):
        b = core // 4
        out[b] += res.results[core]["out"]
    if _trace:
        kernel.last_results = res
    return out


# revision 12
# speedup vs baseline: 1.3995x; 1.0842x over previous
"""Causal multi-head attention (nn_Attention_87840671138123) on 8 trn2 NeuronCores.

Problem (B=2, S=2048, D=1024, H=16 heads, E=64 head_dim), fp32:
    Q = einsum('bsd,hde->bhse', q, W_q)   (same for K, V)
    scores = Q @ K^T / sqrt(D), causal mask, softmax
    attn = probs @ V  -> [B, S, D] (head-major concat)
    out = attn @ W_o.T

Sharding: core = 4*b + quad. Each core handles batch b and a quad of 4 heads
(heads 4*quad .. 4*quad+3). It computes a partial output
    out_part = attn_quad @ W_o.T[quad rows, :]   [S, D]
and the host sums the 4 partials per batch (the "all-reduce" of the output
projection done host-side at gather time).

Device layout choices (per core):
 - Host passes xT = x[b].T  [D, S] so the d-contraction sits on partitions.
 - Projections produce QT/KT in "transposed" layout [head-pair x 64, S]
   (head h2 of a pair occupies partitions 64*h2..64*h2+63), and V in natural
   [t, e] layout augmented with a ones-column (V_aug [t, 65]) so the
   attn matmul also accumulates the softmax denominator as row 64.
 - scoresT[t, s] = (KT chunk).T @ QT  -> exp on ACT (scale 1/32 folded in)
   -> causal handled by (a) skipping fully-masked blocks, (b) shrinking the
   moving dim to the valid s-range for diagonal blocks, (c) one [128,128]
   triangular mask multiply for the diagonal 128-col strip.
 - attnT_aug[65, s] += V_aug.T @ expT accumulated over t chunks in PSUM.
 - Normalize: denom row -> reciprocal -> partition_broadcast -> multiply.
 - Output projection: out[s, :] = sum_g (attnT chunk).T @ W_o.T slice.

Numerics: the Q/K path (projections + scores) runs in bf16 — score errors
are absolute-small (scores ~N(0, 0.1^2)) and only perturb softmax weights,
contributing <~3e-4 relative to the output. The V path (V projection,
attn*V, W_o) stays float32r (tf32-like): value errors there pass straight
through to the output. fp32r also runs at a lower power draw than fp32
HIGH-mode; the all-fp32r version tripped the chip-wide power throttle
(all 8 cores pinned at K=4/8 = 1.2 GHz for ~270us).
"""

import ml_dtypes
import numpy as np

import concourse.bass as bass
import concourse.tile as tile
from concourse import bacc, mybir
from concourse.bass_utils import run_bass_kernel_spmd

B, S, D, H, E = 2, 2048, 1024, 16, 64
P = 128
NCORES = 8
SJ = 512            # s-tile width
NJ = S // SJ        # 4 s-tiles
ND = D // P         # 8 d-chunks
NT = S // P         # 16 t-chunks
f32 = mybir.dt.float32
f32r = mybir.dt.float32r
bf16 = mybir.dt.bfloat16
EXP = mybir.ActivationFunctionType.Exp
MULT = mybir.AluOpType.mult

QK_DT = bf16        # dtype of q/k inputs, Wq/Wk, QT/KT, scores matmul
V_DT = f32r         # dtype of v input, Wv, V_aug, expT, attnG, WoT

_NP_OF = {bf16: ml_dtypes.bfloat16, f32r: np.float32, f32: np.float32}

_NC_CACHE = []


def _build():
    nc = bacc.Bacc("TRN2", target_bir_lowering=False, debug=False)

    qT_d = nc.dram_tensor("qT", [D, S], QK_DT, kind="ExternalInput")
    kT_d = nc.dram_tensor("kT", [D, S], QK_DT, kind="ExternalInput")
    vT_d = nc.dram_tensor("vT", [D, S], V_DT, kind="ExternalInput")
    wq_d = nc.dram_tensor("wq", [D, 4 * E], QK_DT, kind="ExternalInput")
    wk_d = nc.dram_tensor("wk", [D, 4 * E], QK_DT, kind="ExternalInput")
    wv_d = nc.dram_tensor("wv", [D, 4 * E], V_DT, kind="ExternalInput")
    wot_d = nc.dram_tensor("wot", [4 * E, D], V_DT, kind="ExternalInput")
    tri_d = nc.dram_tensor("tri", [P, P], V_DT, kind="ExternalInput")
    out_d = nc.dram_tensor("out", [S, D], f32, kind="ExternalOutput")

    with tile.TileContext(nc) as tc:
        with (
            tc.tile_pool(name="pers", bufs=1) as pers,
            tc.tile_pool(name="xt", bufs=3) as xt_pool,
            tc.tile_pool(name="ex", bufs=3) as ex_pool,
            tc.tile_pool(name="sm", bufs=3) as sm_pool,
            tc.tile_pool(name="ot", bufs=2) as ot_pool,
            tc.tile_pool(name="pj", bufs=3, space="PSUM") as pj_pool,
            tc.tile_pool(name="sc", bufs=3, space="PSUM") as sc_pool,
            tc.tile_pool(name="at", bufs=2, space="PSUM") as at_pool,
        ):
            # ---- persistent weights / constants ----
            wq_sb = pers.tile([P, ND, 4 * E], QK_DT, name="wq_sb")
            wk_sb = pers.tile([P, ND, 4 * E], QK_DT, name="wk_sb")
            wv_sb = pers.tile([P, ND, 4 * E], V_DT, name="wv_sb")
            nc.sync.dma_start(wq_sb[:], wq_d.ap().rearrange("(o p) m -> p o m", p=P))
            nc.sync.dma_start(wk_sb[:], wk_d.ap().rearrange("(o p) m -> p o m", p=P))
            nc.sync.dma_start(wv_sb[:], wv_d.ap().rearrange("(o p) m -> p o m", p=P))
            wot_sb = pers.tile([P, 2, D], V_DT, name="wot_sb")
            nc.sync.dma_start(wot_sb[:], wot_d.ap().rearrange("(g p) n -> p g n", p=P))
            tri_sb = pers.tile([P, P], V_DT, name="tri_sb")
            nc.sync.dma_start(tri_sb[:], tri_d.ap())

            # ---- persistent activations ----
            QT = [pers.tile([P, S], QK_DT, name=f"QT{g}") for g in range(2)]
            # Per-head KT zero-padded to 128 partitions: rows 0..63 hold the
            # head's K^T, rows 64..127 are zeros. The scores matmul then runs
            # with K=128 (full PE rows) -- the zero rows null out the other
            # head's Q rows in the shared QT rhs. Full-array matmuls keep the
            # HAM activity monitor from throttling the PE clock to 1.2 GHz
            # (K=64 / M=65 matmuls read as "half idle").
            KTH = [[pers.tile([P, S], QK_DT, name=f"KT{g}{h2}") for h2 in range(2)]
                   for g in range(2)]
            # V_aug padded to 128 cols: [64 V | ones | 63 zeros] so the attn
            # matmul loads all 128 PE columns (M=128).
            V = [pers.tile([P, NT, 2, P], V_DT, name=f"V{g}") for g in range(2)]
            attnG = [pers.tile([P, S], V_DT, name=f"attnG{g}") for g in range(2)]
            for g in range(2):
                nc.vector.memset(KTH[g][0][E:2 * E, :], 0.0)
                nc.vector.memset(KTH[g][1][0:E, :], 0.0)
                vz_ap = V[g][:, :, :, E + 1:]
                ones_ap = V[g][:, :, :, E:E + 1]
                if V_DT == f32r:
                    vz_ap = vz_ap.bitcast(f32)
                    ones_ap = ones_ap.bitcast(f32)
                nc.vector.memset(vz_ap, 0.0)
                nc.vector.memset(ones_ap, 1.0)

            # ---- phase 1: projections ----
            for j in range(NJ):
                js = slice(j * SJ, (j + 1) * SJ)
                xq = xt_pool.tile([P, ND, SJ], QK_DT, tag="xtq", name=f"xq{j}")
                nc.sync.dma_start(
                    xq[:], qT_d.ap().rearrange("(o p) s -> p o s", p=P)[:, :, js])
                for g in range(2):
                    pq = pj_pool.tile([P, SJ], f32, tag="pj", name=f"pq{j}{g}")
                    for c in range(ND):
                        nc.tensor.matmul(
                            pq[:], wq_sb[:, c, bass.ts(g, P)], xq[:, c, :],
                            start=(c == 0), stop=(c == ND - 1))
                    nc.vector.tensor_copy(QT[g][:, js], pq[:])

                xk = xt_pool.tile([P, ND, SJ], QK_DT, tag="xtq", name=f"xk{j}")
                nc.sync.dma_start(
                    xk[:], kT_d.ap().rearrange("(o p) s -> p o s", p=P)[:, :, js])
                for g in range(2):
                    pk = pj_pool.tile([P, SJ], f32, tag="pj", name=f"pk{j}{g}")
                    for c in range(ND):
                        nc.tensor.matmul(
                            pk[:], wk_sb[:, c, bass.ts(g, P)], xk[:, c, :],
                            start=(c == 0), stop=(c == ND - 1))
                    nc.vector.tensor_copy(KTH[g][0][0:E, js], pk[0:E, :])
                    nc.vector.tensor_copy(
                        KTH[g][1][E:2 * E, js], pk[E:2 * E, :])

                xv = xt_pool.tile([P, ND, SJ], V_DT, tag="xtv", name=f"xv{j}")
                nc.sync.dma_start(
                    xv[:], vT_d.ap().rearrange("(o p) s -> p o s", p=P)[:, :, js])
                for g in range(2):
                    for u in range(SJ // P):
                        t = 4 * j + u
                        pv = pj_pool.tile([P, P], f32, tag="pj", name=f"pv{j}{g}{u}")
                        for c in range(ND):
                            nc.tensor.matmul(
                                pv[:], xv[:, c, bass.ts(u, P)],
                                wv_sb[:, c, bass.ts(g, P)],
                                start=(c == 0), stop=(c == ND - 1))
                        nc.vector.tensor_copy(V[g][:, t, 0, 0:E], pv[:, 0:E])
                        nc.vector.tensor_copy(V[g][:, t, 1, 0:E], pv[:, E:2 * E])

            # ---- phase 2: attention per (pair g, s-tile j) ----
            for g in range(2):
                for j in range(NJ):
                    nblk = 4 * j + 4
                    atp = [
                        at_pool.tile([P, SJ], f32, tag="at", name=f"at{g}{j}{h2}")
                        for h2 in range(2)
                    ]
                    for cb in range(nblk):
                        col0 = max(0, cb - 4 * j) * P
                        # both heads' score matmuls back to back: K=64 row
                        # groups (0,*) and (64,*) run concurrently on the PE
                        scps = []
                        for h2 in range(2):
                            scp = sc_pool.tile(
                                [P, SJ], f32, tag="sc", name=f"sc{g}{j}{cb}{h2}")
                            nc.tensor.matmul(
                                scp[:, col0:],
                                KTH[g][h2][:, bass.ts(cb, P)],
                                QT[g][:, j * SJ + col0:(j + 1) * SJ],
                                start=True, stop=True)
                            scps.append(scp)
                        for h2 in range(2):
                            scp = scps[h2]
                            ex = ex_pool.tile(
                                [P, SJ], V_DT, tag="ex", name=f"ex{g}{j}{cb}{h2}")
                            nc.scalar.activation(
                                ex[:, col0:], scp[:, col0:], EXP, scale=1.0 / 32.0)
                            if cb >= 4 * j:
                                nc.vector.tensor_tensor(
                                    ex[:, col0:col0 + P], ex[:, col0:col0 + P],
                                    tri_sb[:], MULT)
                            nc.tensor.matmul(
                                atp[h2][:, col0:],
                                V[g][:, cb, h2, :],
                                ex[:, col0:],
                                start=(cb == 0), stop=(cb == nblk - 1))
                    # epilogue: normalize by softmax denominator (row E)
                    for h2 in range(2):
                        js = slice(j * SJ, (j + 1) * SJ)
                        den = sm_pool.tile([E + 1, SJ], f32, tag="den",
                                           name=f"den{g}{j}{h2}")
                        nc.vector.tensor_copy(den[E:E + 1, :], atp[h2][E:E + 1, :])
                        rec = sm_pool.tile([1, SJ], f32, tag="rec",
                                           name=f"rec{g}{j}{h2}")
                        nc.sync.dma_start(rec[:], den[E:E + 1, :])
                        nc.vector.reciprocal(rec[:], rec[:])
                        recb = sm_pool.tile([E, SJ], f32, tag="recb",
                                            name=f"recb{g}{j}{h2}")
                        nc.gpsimd.partition_broadcast(recb[:], rec[:])
                        if h2 == 0:
                            nc.vector.tensor_tensor(
                                attnG[g][0:E, js], atp[h2][0:E, :], recb[:], MULT)
                        else:
                            ah = sm_pool.tile([E, SJ], V_DT, tag="ah",
                                              name=f"ah{g}{j}")
                            nc.vector.tensor_tensor(
                                ah[:], atp[h2][0:E, :], recb[:], MULT)
                            nc.sync.dma_start(attnG[g][E:2 * E, js], ah[:])

            # ---- phase 3: output projection (partial over this core's heads) --
            for si in range(NT):
                ot = ot_pool.tile([P, D], f32, tag="ot", name=f"ot{si}")
                for no in range(2):
                    po = pj_pool.tile([P, SJ], f32, tag="pj", name=f"po{si}{no}")
                    for g in range(2):
                        nc.tensor.matmul(
                            po[:], attnG[g][:, bass.ts(si, P)],
                            wot_sb[:, g, bass.ts(no, SJ)],
                            start=(g == 0), stop=(g == 1))
                    nc.vector.tensor_copy(ot[:, bass.ts(no, SJ)], po[:])
                nc.sync.dma_start(out_d.ap()[bass.ts(si, P), :], ot[:])

    nc.compile()
    return nc


def _get_nc():
    if not _NC_CACHE:
        _NC_CACHE.append(_build())
    return _NC_CACHE[0]


def _in_maps(q, k, v, W_q, W_k, W_v, W_o):
    qk_np = _NP_OF[QK_DT]
    v_np = _NP_OF[V_DT]
    tri = (np.arange(P)[:, None] <= np.arange(P)[None, :]).astype(v_np)
    xT = {}
    for b in range(B):
        xT[b] = (
            np.ascontiguousarray(q[b].T).astype(qk_np),
            np.ascontiguousarray(k[b].T).astype(qk_np),
            np.ascontiguousarray(v[b].T).astype(v_np),
        )
    maps = []
    for core in range(NCORES):
        b, quad = divmod(core, 4)
        hs = slice(4 * quad, 4 * quad + 4)
        qT_b, kT_b, vT_b = xT[b]
        maps.append({
            "qT": qT_b,
            "kT": kT_b,
            "vT": vT_b,
            # [4, D, E] -> [D, 4, E] -> [D, 256], col l*64+e = W[4q+l, d, e]
            "wq": np.ascontiguousarray(
                W_q[hs].transpose(1, 0, 2).reshape(D, 4 * E)).astype(qk_np),
            "wk": np.ascontiguousarray(
                W_k[hs].transpose(1, 0, 2).reshape(D, 4 * E)).astype(qk_np),
            "wv": np.ascontiguousarray(
                W_v[hs].transpose(1, 0, 2).reshape(D, 4 * E)).astype(v_np),
            # W_o[out, in] -> W_o.T rows for this quad's 256 input dims
            "wot": np.ascontiguousarray(
                W_o[:, 4 * quad * E:4 * quad * E + 4 * E].T).astype(v_np),
            "tri": tri,
        })
    return maps


def kernel(q, k, v, W_q, W_k, W_v, W_o, _trace=False, _trace_kwargs=None):
    q = np.asarray(q, dtype=np.float32)
    k = np.asarray(k, dtype=np.float32)
    v = np.asarray(v, dtype=np.float32)
    W_q = np.asarray(W_q, dtype=np.float32)
    W_k = np.asarray(W_k, dtype=np.float32)
    W_v = np.asarray(W_v, dtype=np.float32)
    W_o = np.asarray(W_o, dtype=np.float32)

    nc = _get_nc()
    maps = _in_maps(q, k, v, W_q, W_k, W_v, W_o)
    kwargs = dict(_trace_kwargs or {})
    res = run_bass_kernel_spmd(
        nc, maps, core_ids=list(range(NCORES)), trace=_trace, **kwargs)
    out = np.zeros((B, S, D), dtype=np.float32)
    for core in range(NCORES):
        b = core // 4
        out[b] += res.results[core]["out"]
    if _trace:
        kernel.last_results = res
    return out


# revision 13
# speedup vs baseline: 1.6004x; 1.1436x over previous
"""Causal multi-head attention (nn_Attention_87840671138123) on 8 trn2 NeuronCores.

Problem (B=2, S=2048, D=1024, H=16 heads, E=64 head_dim), fp32:
    Q = einsum('bsd,hde->bhse', q, W_q)   (same for K, V)
    scores = Q @ K^T / sqrt(D), causal mask, softmax
    attn = probs @ V  -> [B, S, D] (head-major concat)
    out = attn @ W_o.T

Sharding: core = 4*b + quad. Each core handles batch b and a quad of 4 heads
(heads 4*quad .. 4*quad+3). It computes a partial output
    out_part = attn_quad @ W_o.T[quad rows, :]   [S, D]
and the host sums the 4 partials per batch (the "all-reduce" of the output
projection done host-side at gather time).

Device layout choices (per core):
 - Host passes xT = x[b].T  [D, S] so the d-contraction sits on partitions.
 - Projections produce QT/KT in "transposed" layout [head-pair x 64, S]
   (head h2 of a pair occupies partitions 64*h2..64*h2+63), and V in natural
   [t, e] layout augmented with a ones-column (V_aug [t, 65]) so the
   attn matmul also accumulates the softmax denominator as row 64.
 - scoresT[t, s] = (KT chunk).T @ QT  -> exp on ACT (scale 1/32 folded in)
   -> causal handled by (a) skipping fully-masked blocks, (b) shrinking the
   moving dim to the valid s-range for diagonal blocks, (c) one [128,128]
   triangular mask multiply for the diagonal 128-col strip.
 - attnT_aug[65, s] += V_aug.T @ expT accumulated over t chunks in PSUM.
 - Normalize: denom row -> reciprocal -> partition_broadcast -> multiply.
 - Output projection: out[s, :] = sum_g (attnT chunk).T @ W_o.T slice.

Numerics: the Q/K path (projections + scores) runs in bf16 — score errors
are absolute-small (scores ~N(0, 0.1^2)) and only perturb softmax weights,
contributing <~3e-4 relative to the output. The V path (V projection,
attn*V, W_o) stays float32r (tf32-like): value errors there pass straight
through to the output. fp32r also runs at a lower power draw than fp32
HIGH-mode; the all-fp32r version tripped the chip-wide power throttle
(all 8 cores pinned at K=4/8 = 1.2 GHz for ~270us).
"""

import ml_dtypes
import numpy as np

import concourse.bass as bass
import concourse.tile as tile
from concourse import bacc, mybir
from concourse.bass_utils import run_bass_kernel_spmd

B, S, D, H, E = 2, 2048, 1024, 16, 64
P = 128
NCORES = 8
SJ = 512            # s-tile width
NJ = S // SJ        # 4 s-tiles
ND = D // P         # 8 d-chunks
NT = S // P         # 16 t-chunks
f32 = mybir.dt.float32
f32r = mybir.dt.float32r
bf16 = mybir.dt.bfloat16
fp16 = mybir.dt.float16
EXP = mybir.ActivationFunctionType.Exp
MULT = mybir.AluOpType.mult

QK_DT = bf16        # dtype of q/k inputs, Wq/Wk, QT/KT, scores matmul
V_DT = fp16         # dtype of v input, Wv, V_aug, expT, attnG, WoT

_NP_OF = {bf16: ml_dtypes.bfloat16, fp16: np.float16, f32r: np.float32,
          f32: np.float32}

_NC_CACHE = []


def _build():
    nc = bacc.Bacc("TRN2", target_bir_lowering=False, debug=False)

    qT_d = nc.dram_tensor("qT", [D, S], QK_DT, kind="ExternalInput")
    kT_d = nc.dram_tensor("kT", [D, S], QK_DT, kind="ExternalInput")
    vT_d = nc.dram_tensor("vT", [D, S], V_DT, kind="ExternalInput")
    wq_d = nc.dram_tensor("wq", [D, 4 * E], QK_DT, kind="ExternalInput")
    wk_d = nc.dram_tensor("wk", [D, 4 * E], QK_DT, kind="ExternalInput")
    wv_d = nc.dram_tensor("wv", [D, 4 * E], V_DT, kind="ExternalInput")
    wot_d = nc.dram_tensor("wot", [4 * E, D], V_DT, kind="ExternalInput")
    tri_d = nc.dram_tensor("tri", [P, P], V_DT, kind="ExternalInput")
    out_d = nc.dram_tensor("out", [S, D], f32, kind="ExternalOutput")

    with tile.TileContext(nc) as tc:
        with (
            tc.tile_pool(name="pers", bufs=1) as pers,
            tc.tile_pool(name="xt", bufs=3) as xt_pool,
            tc.tile_pool(name="ex", bufs=3) as ex_pool,
            tc.tile_pool(name="sm", bufs=3) as sm_pool,
            tc.tile_pool(name="ot", bufs=2) as ot_pool,
            tc.tile_pool(name="pj", bufs=3, space="PSUM") as pj_pool,
            tc.tile_pool(name="sc", bufs=3, space="PSUM") as sc_pool,
            tc.tile_pool(name="at", bufs=2, space="PSUM") as at_pool,
        ):
            # ---- persistent weights / constants ----
            wq_sb = pers.tile([P, ND, 4 * E], QK_DT, name="wq_sb")
            wk_sb = pers.tile([P, ND, 4 * E], QK_DT, name="wk_sb")
            wv_sb = pers.tile([P, ND, 4 * E], V_DT, name="wv_sb")
            nc.sync.dma_start(wq_sb[:], wq_d.ap().rearrange("(o p) m -> p o m", p=P))
            nc.sync.dma_start(wk_sb[:], wk_d.ap().rearrange("(o p) m -> p o m", p=P))
            nc.sync.dma_start(wv_sb[:], wv_d.ap().rearrange("(o p) m -> p o m", p=P))
            wot_sb = pers.tile([P, 2, D], V_DT, name="wot_sb")
            nc.sync.dma_start(wot_sb[:], wot_d.ap().rearrange("(g p) n -> p g n", p=P))
            tri_sb = pers.tile([P, P], V_DT, name="tri_sb")
            nc.sync.dma_start(tri_sb[:], tri_d.ap())

            # ---- persistent activations ----
            QT = [pers.tile([P, S], QK_DT, name=f"QT{g}") for g in range(2)]
            # Per-head KT zero-padded to 128 partitions: rows 0..63 hold the
            # head's K^T, rows 64..127 are zeros. The scores matmul then runs
            # with K=128 (full PE rows) -- the zero rows null out the other
            # head's Q rows in the shared QT rhs. Full-array matmuls keep the
            # HAM activity monitor from throttling the PE clock to 1.2 GHz
            # (K=64 / M=65 matmuls read as "half idle").
            KTH = [[pers.tile([P, S], QK_DT, name=f"KT{g}{h2}") for h2 in range(2)]
                   for g in range(2)]
            # V_aug padded to 128 cols: [64 V | ones | 63 zeros] so the attn
            # matmul loads all 128 PE columns (M=128).
            V = [pers.tile([P, NT, 2, P], V_DT, name=f"V{g}") for g in range(2)]
            attnG = [pers.tile([P, S], V_DT, name=f"attnG{g}") for g in range(2)]
            for g in range(2):
                nc.vector.memset(KTH[g][0][E:2 * E, :], 0.0)
                nc.vector.memset(KTH[g][1][0:E, :], 0.0)
                vz_ap = V[g][:, :, :, E + 1:]
                ones_ap = V[g][:, :, :, E:E + 1]
                if V_DT == f32r:
                    vz_ap = vz_ap.bitcast(f32)
                    ones_ap = ones_ap.bitcast(f32)
                nc.vector.memset(vz_ap, 0.0)
                nc.vector.memset(ones_ap, 1.0)

            # ---- phase 1: projections ----
            for j in range(NJ):
                js = slice(j * SJ, (j + 1) * SJ)
                xq = xt_pool.tile([P, ND, SJ], QK_DT, tag="xtq", name=f"xq{j}")
                nc.sync.dma_start(
                    xq[:], qT_d.ap().rearrange("(o p) s -> p o s", p=P)[:, :, js])
                for g in range(2):
                    pq = pj_pool.tile([P, SJ], f32, tag="pj", name=f"pq{j}{g}")
                    for c in range(ND):
                        nc.tensor.matmul(
                            pq[:], wq_sb[:, c, bass.ts(g, P)], xq[:, c, :],
                            start=(c == 0), stop=(c == ND - 1))
                    nc.vector.tensor_copy(QT[g][:, js], pq[:])

                xk = xt_pool.tile([P, ND, SJ], QK_DT, tag="xtq", name=f"xk{j}")
                nc.sync.dma_start(
                    xk[:], kT_d.ap().rearrange("(o p) s -> p o s", p=P)[:, :, js])
                for g in range(2):
                    pk = pj_pool.tile([P, SJ], f32, tag="pj", name=f"pk{j}{g}")
                    for c in range(ND):
                        nc.tensor.matmul(
                            pk[:], wk_sb[:, c, bass.ts(g, P)], xk[:, c, :],
                            start=(c == 0), stop=(c == ND - 1))
                    nc.vector.tensor_copy(KTH[g][0][0:E, js], pk[0:E, :])
                    nc.vector.tensor_copy(
                        KTH[g][1][E:2 * E, js], pk[E:2 * E, :])

                xv = xt_pool.tile([P, ND, SJ], V_DT, tag="xtv", name=f"xv{j}")
                nc.sync.dma_start(
                    xv[:], vT_d.ap().rearrange("(o p) s -> p o s", p=P)[:, :, js])
                for g in range(2):
                    for u in range(SJ // P):
                        t = 4 * j + u
                        pv = pj_pool.tile([P, P], f32, tag="pj", name=f"pv{j}{g}{u}")
                        for c in range(ND):
                            nc.tensor.matmul(
                                pv[:], xv[:, c, bass.ts(u, P)],
                                wv_sb[:, c, bass.ts(g, P)],
                                start=(c == 0), stop=(c == ND - 1))
                        nc.vector.tensor_copy(V[g][:, t, 0, 0:E], pv[:, 0:E])
                        nc.vector.tensor_copy(V[g][:, t, 1, 0:E], pv[:, E:2 * E])

            # ---- phase 2: attention per (pair g, s-tile j) ----
            for g in range(2):
                for j in range(NJ):
                    nblk = 4 * j + 4
                    atp = [
                        at_pool.tile([P, SJ], f32, tag="at", name=f"at{g}{j}{h2}")
                        for h2 in range(2)
                    ]
                    for cb in range(nblk):
                        col0 = max(0, cb - 4 * j) * P
                        # both heads' score matmuls back to back: K=64 row
                        # groups (0,*) and (64,*) run concurrently on the PE
                        scps = []
                        for h2 in range(2):
                            scp = sc_pool.tile(
                                [P, SJ], f32, tag="sc", name=f"sc{g}{j}{cb}{h2}")
                            nc.tensor.matmul(
                                scp[:, col0:],
                                KTH[g][h2][:, bass.ts(cb, P)],
                                QT[g][:, j * SJ + col0:(j + 1) * SJ],
                                start=True, stop=True)
                            scps.append(scp)
                        for h2 in range(2):
                            scp = scps[h2]
                            ex = ex_pool.tile(
                                [P, SJ], V_DT, tag="ex", name=f"ex{g}{j}{cb}{h2}")
                            nc.scalar.activation(
                                ex[:, col0:], scp[:, col0:], EXP, scale=1.0 / 32.0)
                            if cb >= 4 * j:
                                nc.vector.tensor_tensor(
                                    ex[:, col0:col0 + P], ex[:, col0:col0 + P],
                                    tri_sb[:], MULT)
                            nc.tensor.matmul(
                                atp[h2][:, col0:],
                                V[g][:, cb, h2, :],
                                ex[:, col0:],
                                start=(cb == 0), stop=(cb == nblk - 1))
                    # epilogue: normalize by softmax denominator (row E)
                    for h2 in range(2):
                        js = slice(j * SJ, (j + 1) * SJ)
                        den = sm_pool.tile([E + 1, SJ], f32, tag="den",
                                           name=f"den{g}{j}{h2}")
                        nc.vector.tensor_copy(den[E:E + 1, :], atp[h2][E:E + 1, :])
                        rec = sm_pool.tile([1, SJ], f32, tag="rec",
                                           name=f"rec{g}{j}{h2}")
                        nc.sync.dma_start(rec[:], den[E:E + 1, :])
                        nc.vector.reciprocal(rec[:], rec[:])
                        recb = sm_pool.tile([E, SJ], f32, tag="recb",
                                            name=f"recb{g}{j}{h2}")
                        nc.gpsimd.partition_broadcast(recb[:], rec[:])
                        if h2 == 0:
                            nc.vector.tensor_tensor(
                                attnG[g][0:E, js], atp[h2][0:E, :], recb[:], MULT)
                        else:
                            ah = sm_pool.tile([E, SJ], V_DT, tag="ah",
                                              name=f"ah{g}{j}")
                            nc.vector.tensor_tensor(
                                ah[:], atp[h2][0:E, :], recb[:], MULT)
                            nc.sync.dma_start(attnG[g][E:2 * E, js], ah[:])

            # ---- phase 3: output projection (partial over this core's heads) --
            for si in range(NT):
                ot = ot_pool.tile([P, D], f32, tag="ot", name=f"ot{si}")
                for no in range(2):
                    po = pj_pool.tile([P, SJ], f32, tag="pj", name=f"po{si}{no}")
                    for g in range(2):
                        nc.tensor.matmul(
                            po[:], attnG[g][:, bass.ts(si, P)],
                            wot_sb[:, g, bass.ts(no, SJ)],
                            start=(g == 0), stop=(g == 1))
                    nc.vector.tensor_copy(ot[:, bass.ts(no, SJ)], po[:])
                nc.sync.dma_start(out_d.ap()[bass.ts(si, P), :], ot[:])

    nc.compile()
    return nc


def _get_nc():
    if not _NC_CACHE:
        _NC_CACHE.append(_build())
    return _NC_CACHE[0]


def _in_maps(q, k, v, W_q, W_k, W_v, W_o):
    qk_np = _NP_OF[QK_DT]
    v_np = _NP_OF[V_DT]
    tri = (np.arange(P)[:, None] <= np.arange(P)[None, :]).astype(v_np)
    xT = {}
    for b in range(B):
        xT[b] = (
            np.ascontiguousarray(q[b].T).astype(qk_np),
            np.ascontiguousarray(k[b].T).astype(qk_np),
            np.ascontiguousarray(v[b].T).astype(v_np),
        )
    maps = []
    for core in range(NCORES):
        b, quad = divmod(core, 4)
        hs = slice(4 * quad, 4 * quad + 4)
        qT_b, kT_b, vT_b = xT[b]
        maps.append({
            "qT": qT_b,
            "kT": kT_b,
            "vT": vT_b,
            # [4, D, E] -> [D, 4, E] -> [D, 256], col l*64+e = W[4q+l, d, e]
            "wq": np.ascontiguousarray(
                W_q[hs].transpose(1, 0, 2).reshape(D, 4 * E)).astype(qk_np),
            "wk": np.ascontiguousarray(
                W_k[hs].transpose(1, 0, 2).reshape(D, 4 * E)).astype(qk_np),
            "wv": np.ascontiguousarray(
                W_v[hs].transpose(1, 0, 2).reshape(D, 4 * E)).astype(v_np),
            # W_o[out, in] -> W_o.T rows for this quad's 256 input dims
            "wot": np.ascontiguousarray(
                W_o[:, 4 * quad * E:4 * quad * E + 4 * E].T).astype(v_np),
            "tri": tri,
        })
    return maps


def kernel(q, k, v, W_q, W_k, W_v, W_o, _trace=False, _trace_kwargs=None):
    q = np.asarray(q, dtype=np.float32)
    k = np.asarray(k, dtype=np.float32)
    v = np.asarray(v, dtype=np.float32)
    W_q = np.asarray(W_q, dtype=np.float32)
    W_k = np.asarray(W_k, dtype=np.float32)
    W_v = np.asarray(W_v, dtype=np.float32)
    W_o = np.asarray(W_o, dtype=np.float32)

    nc = _get_nc()
    maps = _in_maps(q, k, v, W_q, W_k, W_v, W_o)
    kwargs = dict(_trace_kwargs or {})
    res = run_bass_kernel_spmd(
        nc, maps, core_ids=list(range(NCORES)), trace=_trace, **kwargs)
    out = np.zeros((B, S, D), dtype=np.float32)
    for core in range(NCORES):
        b = core // 4
        out[b] += res.results[core]["out"]
    if _trace:
        kernel.last_results = res
    return out


# revision 14
# speedup vs baseline: 1.6113x; 1.0068x over previous
"""Causal multi-head attention (nn_Attention_87840671138123) on 8 trn2 NeuronCores.

Problem (B=2, S=2048, D=1024, H=16 heads, E=64 head_dim), fp32:
    Q = einsum('bsd,hde->bhse', q, W_q)   (same for K, V)
    scores = Q @ K^T / sqrt(D), causal mask, softmax
    attn = probs @ V  -> [B, S, D] (head-major concat)
    out = attn @ W_o.T

Sharding: core = 4*b + quad. Each core handles batch b and a quad of 4 heads
(heads 4*quad .. 4*quad+3). It computes a partial output
    out_part = attn_quad @ W_o.T[quad rows, :]   [S, D]
and the host sums the 4 partials per batch (the "all-reduce" of the output
projection done host-side at gather time).

Device layout choices (per core):
 - Host passes xT = x[b].T  [D, S] so the d-contraction sits on partitions.
 - Projections produce QT/KT in "transposed" layout [head-pair x 64, S]
   (head h2 of a pair occupies partitions 64*h2..64*h2+63), and V in natural
   [t, e] layout augmented with a ones-column (V_aug [t, 65]) so the
   attn matmul also accumulates the softmax denominator as row 64.
 - scoresT[t, s] = (KT chunk).T @ QT  -> exp on ACT (scale 1/32 folded in)
   -> causal handled by (a) skipping fully-masked blocks, (b) shrinking the
   moving dim to the valid s-range for diagonal blocks, (c) one [128,128]
   triangular mask multiply for the diagonal 128-col strip.
 - attnT_aug[65, s] += V_aug.T @ expT accumulated over t chunks in PSUM.
 - Normalize: denom row -> reciprocal -> partition_broadcast -> multiply.
 - Output projection: out[s, :] = sum_g (attnT chunk).T @ W_o.T slice.

Numerics: the Q/K path (projections + scores) runs in bf16 — score errors
are absolute-small (scores ~N(0, 0.1^2)) and only perturb softmax weights,
contributing <~3e-4 relative to the output. The V path (V projection,
attn*V, W_o) stays float32r (tf32-like): value errors there pass straight
through to the output. fp32r also runs at a lower power draw than fp32
HIGH-mode; the all-fp32r version tripped the chip-wide power throttle
(all 8 cores pinned at K=4/8 = 1.2 GHz for ~270us).
"""

import ml_dtypes
import numpy as np

import concourse.bass as bass
import concourse.tile as tile
from concourse import bacc, mybir
from concourse.bass_utils import run_bass_kernel_spmd

B, S, D, H, E = 2, 2048, 1024, 16, 64
P = 128
NCORES = 8
SJ = 512            # s-tile width
NJ = S // SJ        # 4 s-tiles
ND = D // P         # 8 d-chunks
NT = S // P         # 16 t-chunks
f32 = mybir.dt.float32
f32r = mybir.dt.float32r
bf16 = mybir.dt.bfloat16
fp16 = mybir.dt.float16
EXP = mybir.ActivationFunctionType.Exp
MULT = mybir.AluOpType.mult

QK_DT = bf16        # dtype of q/k inputs, Wq/Wk, QT/KT, scores matmul
V_DT = fp16         # dtype of v input, Wv, V_aug, expT, attnG, WoT

_NP_OF = {bf16: ml_dtypes.bfloat16, fp16: np.float16, f32r: np.float32,
          f32: np.float32}

_NC_CACHE = []


def _build():
    nc = bacc.Bacc("TRN2", target_bir_lowering=False, debug=False)

    qT_d = nc.dram_tensor("qT", [D, S], QK_DT, kind="ExternalInput")
    kT_d = nc.dram_tensor("kT", [D, S], QK_DT, kind="ExternalInput")
    vT_d = nc.dram_tensor("vT", [D, S], V_DT, kind="ExternalInput")
    wq_d = nc.dram_tensor("wq", [D, 4 * E], QK_DT, kind="ExternalInput")
    wk_d = nc.dram_tensor("wk", [D, 4 * E], QK_DT, kind="ExternalInput")
    wv_d = nc.dram_tensor("wv", [D, 4 * E], V_DT, kind="ExternalInput")
    wot_d = nc.dram_tensor("wot", [4 * E, D], V_DT, kind="ExternalInput")
    tri_d = nc.dram_tensor("tri", [P, P], V_DT, kind="ExternalInput")
    out_d = nc.dram_tensor("out", [S, D], f32, kind="ExternalOutput")

    with tile.TileContext(nc) as tc:
        with (
            tc.tile_pool(name="pers", bufs=1) as pers,
            tc.tile_pool(name="xt", bufs=3) as xt_pool,
            tc.tile_pool(name="ex", bufs=3) as ex_pool,
            tc.tile_pool(name="sm", bufs=3) as sm_pool,
            tc.tile_pool(name="ot", bufs=2) as ot_pool,
            tc.tile_pool(name="pj", bufs=3, space="PSUM") as pj_pool,
            tc.tile_pool(name="sc", bufs=3, space="PSUM") as sc_pool,
            tc.tile_pool(name="at", bufs=2, space="PSUM") as at_pool,
        ):
            # ---- persistent weights / constants ----
            wq_sb = pers.tile([P, ND, 4 * E], QK_DT, name="wq_sb")
            wk_sb = pers.tile([P, ND, 4 * E], QK_DT, name="wk_sb")
            wv_sb = pers.tile([P, ND, 4 * E], V_DT, name="wv_sb")
            nc.sync.dma_start(wq_sb[:], wq_d.ap().rearrange("(o p) m -> p o m", p=P))
            nc.sync.dma_start(wk_sb[:], wk_d.ap().rearrange("(o p) m -> p o m", p=P))
            nc.sync.dma_start(wv_sb[:], wv_d.ap().rearrange("(o p) m -> p o m", p=P))
            wot_sb = pers.tile([P, 2, D], V_DT, name="wot_sb")
            nc.sync.dma_start(wot_sb[:], wot_d.ap().rearrange("(g p) n -> p g n", p=P))
            tri_sb = pers.tile([P, P], V_DT, name="tri_sb")
            nc.sync.dma_start(tri_sb[:], tri_d.ap())

            # ---- persistent activations ----
            QT = [pers.tile([P, S], QK_DT, name=f"QT{g}") for g in range(2)]
            # Per-head KT zero-padded to 128 partitions: rows 0..63 hold the
            # head's K^T, rows 64..127 are zeros. The scores matmul then runs
            # with K=128 (full PE rows) -- the zero rows null out the other
            # head's Q rows in the shared QT rhs. Full-array matmuls keep the
            # HAM activity monitor from throttling the PE clock to 1.2 GHz
            # (K=64 / M=65 matmuls read as "half idle").
            KTH = [[pers.tile([P, S], QK_DT, name=f"KT{g}{h2}") for h2 in range(2)]
                   for g in range(2)]
            # V_aug padded to 128 cols: [64 V | ones | 63 zeros] so the attn
            # matmul loads all 128 PE columns (M=128).
            V = [pers.tile([P, NT, 2, P], V_DT, name=f"V{g}") for g in range(2)]
            attnG = [pers.tile([P, S], V_DT, name=f"attnG{g}") for g in range(2)]
            for g in range(2):
                # pad rows hold ~1e-20: real bit-switching for the HAM
                # activity monitor, but adds only ~1e-18 to each score
                nc.vector.memset(KTH[g][0][E:2 * E, :], 1e-20)
                nc.vector.memset(KTH[g][1][0:E, :], 1e-20)
                vz_ap = V[g][:, :, :, E + 1:]
                ones_ap = V[g][:, :, :, E:E + 1]
                if V_DT == f32r:
                    vz_ap = vz_ap.bitcast(f32)
                    ones_ap = ones_ap.bitcast(f32)
                # pad cols only feed unused PSUM rows 65..127 -> any value;
                # 1.0 keeps the PE array switching (HAM activity)
                nc.vector.memset(vz_ap, 1.0)
                nc.vector.memset(ones_ap, 1.0)

            # ---- phase 1: projections ----
            for j in range(NJ):
                js = slice(j * SJ, (j + 1) * SJ)
                xq = xt_pool.tile([P, ND, SJ], QK_DT, tag="xtq", name=f"xq{j}")
                nc.sync.dma_start(
                    xq[:], qT_d.ap().rearrange("(o p) s -> p o s", p=P)[:, :, js])
                for g in range(2):
                    pq = pj_pool.tile([P, SJ], f32, tag="pj", name=f"pq{j}{g}")
                    for c in range(ND):
                        nc.tensor.matmul(
                            pq[:], wq_sb[:, c, bass.ts(g, P)], xq[:, c, :],
                            start=(c == 0), stop=(c == ND - 1))
                    nc.vector.tensor_copy(QT[g][:, js], pq[:])

                xk = xt_pool.tile([P, ND, SJ], QK_DT, tag="xtq", name=f"xk{j}")
                nc.sync.dma_start(
                    xk[:], kT_d.ap().rearrange("(o p) s -> p o s", p=P)[:, :, js])
                for g in range(2):
                    pk = pj_pool.tile([P, SJ], f32, tag="pj", name=f"pk{j}{g}")
                    for c in range(ND):
                        nc.tensor.matmul(
                            pk[:], wk_sb[:, c, bass.ts(g, P)], xk[:, c, :],
                            start=(c == 0), stop=(c == ND - 1))
                    nc.vector.tensor_copy(KTH[g][0][0:E, js], pk[0:E, :])
                    nc.vector.tensor_copy(
                        KTH[g][1][E:2 * E, js], pk[E:2 * E, :])

                xv = xt_pool.tile([P, ND, SJ], V_DT, tag="xtv", name=f"xv{j}")
                nc.sync.dma_start(
                    xv[:], vT_d.ap().rearrange("(o p) s -> p o s", p=P)[:, :, js])
                for g in range(2):
                    for u in range(SJ // P):
                        t = 4 * j + u
                        pv = pj_pool.tile([P, P], f32, tag="pj", name=f"pv{j}{g}{u}")
                        for c in range(ND):
                            nc.tensor.matmul(
                                pv[:], xv[:, c, bass.ts(u, P)],
                                wv_sb[:, c, bass.ts(g, P)],
                                start=(c == 0), stop=(c == ND - 1))
                        nc.vector.tensor_copy(V[g][:, t, 0, 0:E], pv[:, 0:E])
                        nc.vector.tensor_copy(V[g][:, t, 1, 0:E], pv[:, E:2 * E])

            # ---- phase 2: attention per (pair g, s-tile j) ----
            for g in range(2):
                for j in range(NJ):
                    nblk = 4 * j + 4
                    atp = [
                        at_pool.tile([P, SJ], f32, tag="at", name=f"at{g}{j}{h2}")
                        for h2 in range(2)
                    ]
                    for cb in range(nblk):
                        col0 = max(0, cb - 4 * j) * P
                        # both heads' score matmuls back to back: K=64 row
                        # groups (0,*) and (64,*) run concurrently on the PE
                        scps = []
                        for h2 in range(2):
                            scp = sc_pool.tile(
                                [P, SJ], f32, tag="sc", name=f"sc{g}{j}{cb}{h2}")
                            nc.tensor.matmul(
                                scp[:, col0:],
                                KTH[g][h2][:, bass.ts(cb, P)],
                                QT[g][:, j * SJ + col0:(j + 1) * SJ],
                                start=True, stop=True)
                            scps.append(scp)
                        for h2 in range(2):
                            scp = scps[h2]
                            ex = ex_pool.tile(
                                [P, SJ], V_DT, tag="ex", name=f"ex{g}{j}{cb}{h2}")
                            nc.scalar.activation(
                                ex[:, col0:], scp[:, col0:], EXP, scale=1.0 / 32.0)
                            if cb >= 4 * j:
                                nc.vector.tensor_tensor(
                                    ex[:, col0:col0 + P], ex[:, col0:col0 + P],
                                    tri_sb[:], MULT)
                            nc.tensor.matmul(
                                atp[h2][:, col0:],
                                V[g][:, cb, h2, :],
                                ex[:, col0:],
                                start=(cb == 0), stop=(cb == nblk - 1))
                    # epilogue: normalize by softmax denominator (row E)
                    for h2 in range(2):
                        js = slice(j * SJ, (j + 1) * SJ)
                        den = sm_pool.tile([E + 1, SJ], f32, tag="den",
                                           name=f"den{g}{j}{h2}")
                        nc.vector.tensor_copy(den[E:E + 1, :], atp[h2][E:E + 1, :])
                        rec = sm_pool.tile([1, SJ], f32, tag="rec",
                                           name=f"rec{g}{j}{h2}")
                        nc.sync.dma_start(rec[:], den[E:E + 1, :])
                        nc.vector.reciprocal(rec[:], rec[:])
                        recb = sm_pool.tile([E, SJ], f32, tag="recb",
                                            name=f"recb{g}{j}{h2}")
                        nc.gpsimd.partition_broadcast(recb[:], rec[:])
                        if h2 == 0:
                            nc.vector.tensor_tensor(
                                attnG[g][0:E, js], atp[h2][0:E, :], recb[:], MULT)
                        else:
                            ah = sm_pool.tile([E, SJ], V_DT, tag="ah",
                                              name=f"ah{g}{j}")
                            nc.vector.tensor_tensor(
                                ah[:], atp[h2][0:E, :], recb[:], MULT)
                            nc.sync.dma_start(attnG[g][E:2 * E, js], ah[:])

            # ---- phase 3: output projection (partial over this core's heads) --
            for si in range(NT):
                ot = ot_pool.tile([P, D], f32, tag="ot", name=f"ot{si}")
                for no in range(2):
                    po = pj_pool.tile([P, SJ], f32, tag="pj", name=f"po{si}{no}")
                    for g in range(2):
                        nc.tensor.matmul(
                            po[:], attnG[g][:, bass.ts(si, P)],
                            wot_sb[:, g, bass.ts(no, SJ)],
                            start=(g == 0), stop=(g == 1))
                    nc.vector.tensor_copy(ot[:, bass.ts(no, SJ)], po[:])
                nc.sync.dma_start(out_d.ap()[bass.ts(si, P), :], ot[:])

    nc.compile()
    return nc


def _get_nc():
    if not _NC_CACHE:
        _NC_CACHE.append(_build())
    return _NC_CACHE[0]


def _in_maps(q, k, v, W_q, W_k, W_v, W_o):
    qk_np = _NP_OF[QK_DT]
    v_np = _NP_OF[V_DT]
    tri = (np.arange(P)[:, None] <= np.arange(P)[None, :]).astype(v_np)
    xT = {}
    for b in range(B):
        xT[b] = (
            np.ascontiguousarray(q[b].T).astype(qk_np),
            np.ascontiguousarray(k[b].T).astype(qk_np),
            np.ascontiguousarray(v[b].T).astype(v_np),
        )
    maps = []
    for core in range(NCORES):
        b, quad = divmod(core, 4)
        hs = slice(4 * quad, 4 * quad + 4)
        qT_b, kT_b, vT_b = xT[b]
        maps.append({
            "qT": qT_b,
            "kT": kT_b,
            "vT": vT_b,
            # [4, D, E] -> [D, 4, E] -> [D, 256], col l*64+e = W[4q+l, d, e]
            "wq": np.ascontiguousarray(
                W_q[hs].transpose(1, 0, 2).reshape(D, 4 * E)).astype(qk_np),
            "wk": np.ascontiguousarray(
                W_k[hs].transpose(1, 0, 2).reshape(D, 4 * E)).astype(qk_np),
            "wv": np.ascontiguousarray(
                W_v[hs].transpose(1, 0, 2).reshape(D, 4 * E)).astype(v_np),
            # W_o[out, in] -> W_o.T rows for this quad's 256 input dims
            "wot": np.ascontiguousarray(
                W_o[:, 4 * quad * E:4 * quad * E + 4 * E].T).astype(v_np),
            "tri": tri,
        })
    return maps


def kernel(q, k, v, W_q, W_k, W_v, W_o, _trace=False, _trace_kwargs=None):
    q = np.asarray(q, dtype=np.float32)
    k = np.asarray(k, dtype=np.float32)
    v = np.asarray(v, dtype=np.float32)
    W_q = np.asarray(W_q, dtype=np.float32)
    W_k = np.asarray(W_k, dtype=np.float32)
    W_v = np.asarray(W_v, dtype=np.float32)
    W_o = np.asarray(W_o, dtype=np.float32)

    nc = _get_nc()
    maps = _in_maps(q, k, v, W_q, W_k, W_v, W_o)
    kwargs = dict(_trace_kwargs or {})
    res = run_bass_kernel_spmd(
        nc, maps, core_ids=list(range(NCORES)), trace=_trace, **kwargs)
    out = np.zeros((B, S, D), dtype=np.float32)
    for core in range(NCORES):
        b = core // 4
        out[b] += res.results[core]["out"]
    if _trace:
        kernel.last_results = res
    return out
